# revision 24
# baseline (speedup 1.0000x reference)
"""C3DLoss kernel for Trainium2 — 8-core batch-parallel, raw-Bass implementation.

Per core = one batch frame b (tgt pairing partner tb = b^1):
    partial = sum over both terms (same-frame, cross-frame), all 25 shifts
              delta in [-2,2]^2, all pixels p of
        mref(p) * mq(p+delta) * exp(-50*(|xyz_r(p)-xyz_q(p+d)|^2
                                         + |rgb_r(p)-rgb_q(p+d)|^2))
    loss = -(sum of partials) / max(sum(depth_gt_mask), 1)

Device mapping (v5 — fp16 hybrid: squared-diff xyz + dot-expansion rgb):
  - |rgb_r - rgb_q|^2 is expanded as R2 + Q2 - 2*R.Q so the rgb side needs
    only a PRODUCT per shift (no subtract+square): cancellation is safe
    because rgb magnitudes are <=1 (fp16 products quantize to ~2e-4).
    xyz (magnitudes up to ~10) stays on the subtract-then-square path.
  - Host pre-blocks planes into G=32 W-blocks with +-2 halo ([G, Hp, WBH],
    fp16).  Partitions = (channel, block); shifts are flat free-dim offsets.
  - Channels (all fp16; per term t):
      A tile (sq-diff, sel weight +1):
        ref   [xg, yg, zg, sqrt(R2c + 400*(1-mg))]
        query [xq, yq, zq, 0]
      B tile (dot, sel weights -2,-2,-2,+1):
        ref   [r, g, b, 1]
        query [r, g, b, Q2c + 400*(1-mq)]   (halo fill 400 kills oob pairs)
    PSUM accumulates  |dxyz|^2 + R2c+400(1-mg) - 2*R.Q + Q2c+400(1-mq)
      = d2 + masks;  ACT applies exp(-50 * psum) with accum_out.
  - Engine split per (term,shift) x 32-row slab [flat free = 1344]:
      DVE:    dA = rA - qA[off] (all slots), prodB for slots 0..nb-3
      GPSIMD: prodB for the last 2 slots of each batch
      ACT:    sqA = Square(dA) (fp16), exp chunks (junk out -> PSUM bank 7)
      PE:     per chunk: selA@sqA (start) + selB@prodB (stop) into
              32-partition PSUM slots, halo cols skipped by strided rhs
  - Batch-granular semaphores (4 shift-slots per sync group) keep each
    engine's ops back-to-back; per-instruction sem updates measurably
    serialize the DVE pipeline.  SBUF bandwidth is the binding resource:
    fp16 inputs + the dot trick cut traffic from ~45KB to ~27KB per
    (slot,slab) per partition.
"""

import sys

for _p in ("/opt/trn_rl_repo", "/opt/pypackages"):
    if _p not in sys.path:
        sys.path.insert(0, _p)

from contextlib import ExitStack

import numpy as np

import concourse.bass as bass
import concourse.mybir as mybir
from concourse.ap import AP
from concourse.alu_op_type import AluOpType

F32 = mybir.dt.float32
F16 = mybir.dt.float16

R = 2
G = 32           # W-blocks; one shift-slot = 32 partitions (PE quadrant)
CA = 4           # tile A channels: x, y, z, sqrt(LF)
CB = 4           # tile B channels: r, g, b, LG
SBATCH = 4       # shift slots per 128-partition PSUM bank
NPSUM = 7        # rotating PSUM banks (8th bank = exp junk output)
NSQ = 12         # rotating sq/prod buffers (per side)
NDA = 12         # rotating diff buffers
MSK_C = 400.0
EXP_SCALE = -50.0


class Cfg:
    def __init__(self, H=352, W=1216, HS=32):
        assert W % G == 0 and H % HS == 0
        self.H, self.W, self.HS = H, W, HS
        self.WB = W // G
        self.WBH = self.WB + 2 * R
        self.Hp = H + 2 * R
        self.NSLAB = H // HS
        self.NQ = G * self.Hp * self.WBH     # haloed plane elems
        self.QF = (HS + 2 * R) * self.WBH    # query tile free size (1512)
        self.SF = HS * self.WBH              # slab tile free size (1344)
        cr = max(1, 512 // self.WB)
        self.rchunks = []
        o = 0
        while o < HS:
            self.rchunks.append((o, min(cr, HS - o)))
            o += cr
        self.NC = len(self.rchunks)          # 3 (13,13,6 rows)
        self.slots = [(t, dy, dx) for t in (0, 1)
                      for dy in range(-R, R + 1) for dx in range(-R, R + 1)]
        self.NSLOT = len(self.slots)         # 50
        self.batches = [self.slots[i:i + SBATCH]
                        for i in range(0, len(self.slots), SBATCH)]
        self.NB = len(self.batches)          # 13
        self.UPS = self.NB * self.NC         # units per slab (39)
        self.n_acc = self.NSLAB * self.UPS   # 429

    def batch_of_slot(self, Jg):
        return (Jg // self.NSLOT) * self.NB + (Jg % self.NSLOT) // SBATCH

    def slot_end(self, s, b):
        return s * self.NSLOT + min(SBATCH * (b + 1), self.NSLOT)


def _apv(t_ap, p0, pcnt, free_dims, free_off=0):
    pstride = t_ap.ap[0][0]
    base = t_ap.offset + p0 * pstride + free_off
    return AP(t_ap.tensor, base, [[pstride, pcnt]] + [list(d) for d in free_dims])


def _dram_ap(handle, offset, dims):
    a = handle[:]
    return AP(a.tensor, a.offset + offset, [list(d) for d in dims])


def make_selA():
    s = np.zeros((CA * G, G), dtype=np.float16)
    for c in range(CA):
        for g in range(G):
            s[c * G + g, g] = 1.0
    return s


def make_selB():
    s = np.zeros((CB * G, G), dtype=np.float16)
    for c in range(CB):
        w = -2.0 if c < 3 else 1.0
        for g in range(G):
            s[c * G + g, g] = w
    return s


def emit(nc: bass.Bass, cfg: Cfg):
    HS, WB, WBH = cfg.HS, cfg.WB, cfg.WBH
    NQ, QF, SF = cfg.NQ, cfg.QF, cfg.SF
    NSLAB, NB, NC = cfg.NSLAB, cfg.NB, cfg.NC
    NSLOT = cfg.NSLOT
    Act = mybir.ActivationFunctionType

    dp = nc.declare_dram_parameter
    qa_d = dp("qa_d", [2, CA, NQ], F16, isOutput=False)   # query A (haloed)
    ra_d = dp("ra_d", [2, CA, NQ], F16, isOutput=False)   # ref A (haloed)
    qb_d = dp("qb_d", [2, CB, NQ], F16, isOutput=False)   # query B (haloed)
    rb_d = dp("rb_d", [2, CB, NQ], F16, isOutput=False)   # ref B (haloed)
    selA_d = dp("selA_d", [CA * G, G], F16, isOutput=False)
    selB_d = dp("selB_d", [CB * G, G], F16, isOutput=False)
    out_d = dp("out_d", [128, 1], F32, isOutput=True)
    dbg_d = dp("dbg_d", [128, cfg.n_acc], F32, isOutput=True)

    LD = 8            # load DMAs per slab

    def unit(s, b, c):
        return s * cfg.UPS + b * NC + c

    with ExitStack() as ex:
        E = ex.enter_context
        qa_s = [[E(nc.sbuf_tensor(f"qa{t}{p}", [CA * G, QF + 4], F16))
                 for p in range(3)] for t in range(2)]
        ra_s = [[E(nc.sbuf_tensor(f"ra{t}{p}", [CA * G, SF], F16))
                 for p in range(3)] for t in range(2)]
        qb_s = [[E(nc.sbuf_tensor(f"qb{t}{p}", [CB * G, QF + 4], F16))
                 for p in range(3)] for t in range(2)]
        rb_s = [[E(nc.sbuf_tensor(f"rb{t}{p}", [CB * G, SF], F16))
                 for p in range(3)] for t in range(2)]
        da_t = E(nc.sbuf_tensor("dat", [CA * G, NDA * SF], F16))
        sq_t = E(nc.sbuf_tensor("sqt", [CA * G, NSQ * SF], F16))
        pr_t = E(nc.sbuf_tensor("prt", [CB * G, NSQ * SF], F16))
        acc_s = E(nc.sbuf_tensor("acc", [128, cfg.n_acc], F32))
        res_s = E(nc.sbuf_tensor("res", [128, 1], F32))
        selA_s = E(nc.sbuf_tensor("selA", [CA * G, G], F16))
        selB_s = E(nc.sbuf_tensor("selB", [CB * G, G], F16))
        ps_s = [E(nc.psum_tensor(f"ps{i}", [128, 512], F32))
                for i in range(NPSUM)]
        kt_ps = E(nc.psum_tensor("ktps", [128, 512], F32))

        sLC = E(nc.semaphore("sLC"))  # constant loads
        sL0 = E(nc.semaphore("sL0"))  # even-slab loads
        sL1 = E(nc.semaphore("sL1"))  # odd-slab loads
        sL2 = E(nc.semaphore("sL2"))  # third-phase loads
        sG = E(nc.semaphore("sG"))   # gpsimd memset done
        sV = E(nc.semaphore("sV"))    # DVE batch group done (subs+sq+mults)
        sVs = E(nc.semaphore("sVs"))  # DVE subs done per batch
        sF = E(nc.semaphore("sF"))    # DVE final reduce
        sA1 = E(nc.semaphore("sA1"))  # ACT sq group per batch
        sP = E(nc.semaphore("sP"))   # PE per unit
        sA = E(nc.semaphore("sA"))   # ACT exp per unit
        sL = E(nc.semaphore("sL"))   # final output DMAs
        blk = E(nc.Block())

        @blk.gpsimd
        def _(gp):
            gp.memset(acc_s.ap(), 0.0)
            gp.memset(res_s.ap(), 0.0)
            gp.drain()
            gp.sem_inc(sG, 8)

        @blk.sync
        def _(sp):
            sp.dma_start(selA_s[:], selA_d[:]).then_inc(sLC, 16)
            sp.dma_start(selB_s[:], selB_d[:]).then_inc(sLC, 16)
            for s in range(NSLAB):
                ph = s % 3
                if s >= 3:
                    sp.wait_ge(sV, NB * (s - 2))
                r0 = s * HS
                sLs = (sL0, sL1, sL2)[s % 3]
                for t in range(2):
                    sp.dma_start(
                        _apv(qa_s[t][ph].ap(), 0, CA * G, [[1, QF]], 2),
                        _dram_ap(qa_d, t * CA * NQ + r0 * WBH,
                                 [[NQ, CA], [cfg.Hp * WBH, G], [1, QF]])
                    ).then_inc(sLs, 16)
                    sp.dma_start(
                        ra_s[t][ph].ap(),
                        _dram_ap(ra_d, t * CA * NQ + (r0 + R) * WBH,
                                 [[NQ, CA], [cfg.Hp * WBH, G], [1, SF]])
                    ).then_inc(sLs, 16)
                    sp.dma_start(
                        _apv(qb_s[t][ph].ap(), 0, CB * G, [[1, QF]], 2),
                        _dram_ap(qb_d, t * CB * NQ + r0 * WBH,
                                 [[NQ, CB], [cfg.Hp * WBH, G], [1, QF]])
                    ).then_inc(sLs, 16)
                    sp.dma_start(
                        rb_s[t][ph].ap(),
                        _dram_ap(rb_d, t * CB * NQ + (r0 + R) * WBH,
                                 [[NQ, CB], [cfg.Hp * WBH, G], [1, SF]])
                    ).then_inc(sLs, 16)
            # final output
            sp.wait_ge(sF, 1)
            sp.dma_start(out_d[:], res_s.ap()).then_inc(sL, 16)
            sp.dma_start(dbg_d[:], acc_s.ap()).then_inc(sL, 16)

        @blk.vector
        def _(ve):
            for s in range(NSLAB):
                ph = s % 3
                sLs = (sL0, sL1, sL2)[s % 3]
                ve.wait_ge(sLs, 16 * LD * (s // 3 + 1))
                for b, bslots in enumerate(cfg.batches):
                    nb = len(bslots)
                    lastJ = cfg.slot_end(s, b) - 1
                    if lastJ >= NDA:
                        bold = cfg.batch_of_slot(lastJ - NDA)
                        ve.wait_ge(sA1, bold + 1)
                    if lastJ >= NSQ:
                        bold = cfg.batch_of_slot(lastJ - NSQ)
                        ve.wait_ge(sP, NC * (bold + 1))
                    J0 = s * NSLOT + b * SBATCH
                    Bg2 = s * NB + b
                    # greedy same-term pairs of consecutive slots
                    prs = []
                    i = 0
                    while i < nb:
                        if (i + 1 < nb and bslots[i][0] == bslots[i + 1][0]
                                and (J0 + i) % NDA != NDA - 1
                                and (J0 + i) % NSQ != NSQ - 1):
                            prs.append((i, 2))
                            i += 2
                        else:
                            prs.append((i, 1))
                            i += 1
                    offs = [2 + (R + dy) * WBH + dx for (_, dy, dx) in bslots]
                    last = None
                    for (i, cnt) in prs:
                        t = bslots[i][0]
                        Jg = J0 + i
                        if cnt == 2:
                            dlt = offs[i + 1] - offs[i]
                            last = nc.vector.tensor_tensor(
                                _apv(da_t.ap(), 0, CA * G,
                                     [[SF, 2], [1, SF]], (Jg % NDA) * SF),
                                _apv(ra_s[t][ph].ap(), 0, CA * G,
                                     [[0, 2], [1, SF]]),
                                _apv(qa_s[t][ph].ap(), 0, CA * G,
                                     [[dlt, 2], [1, SF]], offs[i]),
                                AluOpType.subtract)
                        else:
                            last = nc.vector.tensor_tensor(
                                _apv(da_t.ap(), 0, CA * G,
                                     [[1, SF]], (Jg % NDA) * SF),
                                ra_s[t][ph].ap(),
                                _apv(qa_s[t][ph].ap(), 0, CA * G,
                                     [[1, SF]], offs[i]),
                                AluOpType.subtract)
                    last.then_inc(sVs, 1)
                    if Bg2 % 5 == 0 and nb == SBATCH:
                        nc.vector.tensor_mul(
                            _apv(sq_t.ap(), 0, CA * G,
                                 [[SF, 2], [1, SF]], (J0 % NSQ) * SF),
                            _apv(da_t.ap(), 0, CA * G,
                                 [[SF, 2], [1, SF]], (J0 % NDA) * SF),
                            _apv(da_t.ap(), 0, CA * G,
                                 [[SF, 2], [1, SF]], (J0 % NDA) * SF))
                    for (i, cnt) in prs:
                        t = bslots[i][0]
                        Jg = J0 + i
                        if cnt == 2:
                            dlt = offs[i + 1] - offs[i]
                            last = nc.vector.tensor_mul(
                                _apv(pr_t.ap(), 0, CB * G,
                                     [[SF, 2], [1, SF]], (Jg % NSQ) * SF),
                                _apv(rb_s[t][ph].ap(), 0, CB * G,
                                     [[0, 2], [1, SF]]),
                                _apv(qb_s[t][ph].ap(), 0, CB * G,
                                     [[dlt, 2], [1, SF]], offs[i]))
                        else:
                            last = nc.vector.tensor_mul(
                                _apv(pr_t.ap(), 0, CB * G,
                                     [[1, SF]], (Jg % NSQ) * SF),
                                rb_s[t][ph].ap(),
                                _apv(qb_s[t][ph].ap(), 0, CB * G,
                                     [[1, SF]], offs[i]))
                    last.then_inc(sV, 1)
            # final reduction of acc columns
            ve.wait_ge(sA, cfg.n_acc)
            nc.vector.tensor_reduce(
                res_s.ap(), acc_s.ap(), axis=mybir.AxisListType.X,
                op=AluOpType.add).then_inc(sF, 1)

        @blk.tensor
        def _(pe):
            pe.wait_ge(sLC, 32)
            for s in range(NSLAB):
                for b, bslots in enumerate(cfg.batches):
                    Bg = s * NB + b
                    pe.wait_ge(sV, Bg + 1)
                    pe.wait_ge(sA1, Bg + 1)
                    for c, (ro, nr) in enumerate(cfg.rchunks):
                        u = unit(s, b, c)
                        if u >= NPSUM:
                            pe.wait_ge(sA, u - NPSUM + 1)
                        pt = ps_s[u % NPSUM]
                        cn = nr * WB
                        for j in range(len(bslots)):
                            Jg = s * NSLOT + b * SBATCH + j
                            last = (j == len(bslots) - 1)
                            nc.tensor.matmul(
                                pt[G * j:G * (j + 1), :cn], selA_s[:],
                                _apv(sq_t.ap(), 0, CA * G,
                                     [[WBH, nr], [1, WB]],
                                     (Jg % NSQ) * SF + ro * WBH + R),
                                start=True, stop=False, skip_group_check=True,
                                tile_position=(0, G * j))
                            mm = nc.tensor.matmul(
                                pt[G * j:G * (j + 1), :cn], selB_s[:],
                                _apv(pr_t.ap(), 0, CB * G,
                                     [[WBH, nr], [1, WB]],
                                     (Jg % NSQ) * SF + ro * WBH + R),
                                start=False, stop=True, skip_group_check=True,
                                tile_position=(0, G * j))
                            if last:
                                mm.then_inc(sP, 1)

        @blk.scalar
        def _(ac):
            ac.wait_ge(sG, 1)
            for s in range(NSLAB):
                for b in range(NB):
                    Bg = s * NB + b
                    lastJ = cfg.slot_end(s, b) - 1
                    if lastJ >= NSQ:
                        bold = cfg.batch_of_slot(lastJ - NSQ)
                        ac.wait_ge(sP, NC * (bold + 1))
                    ac.wait_ge(sVs, Bg + 1)
                    J0 = s * NSLOT + b * SBATCH
                    nbt = len(cfg.batches[b])
                    sq_pairs = []
                    i = 0
                    while i < nbt:
                        if (i + 1 < nbt and (J0 + i) % NDA != NDA - 1
                                and (J0 + i) % NSQ != NSQ - 1):
                            sq_pairs.append((J0 + i, 2))
                            i += 2
                        else:
                            sq_pairs.append((J0 + i, 1))
                            i += 1
                    if Bg % 5 == 0 and nbt == SBATCH:
                        sq_pairs = sq_pairs[1:]   # DVE took the first pair
                    for pi, (Jg, pc) in enumerate(sq_pairs):
                        mm = nc.scalar.square(
                            _apv(sq_t.ap(), 0, CA * G,
                                 [[SF, pc], [1, SF]], (Jg % NSQ) * SF),
                            _apv(da_t.ap(), 0, CA * G,
                                 [[SF, pc], [1, SF]], (Jg % NDA) * SF))
                        if pi == len(sq_pairs) - 1:
                            mm.then_inc(sA1, 1)
                    # exps for the previous batch
                    bprev = b - 1
                    sprev = s
                    if b == 0:
                        sprev, bprev = s - 1, NB - 1
                    if sprev >= 0:
                        pb = G * len(cfg.batches[bprev])
                        for c, (ro, nr) in enumerate(cfg.rchunks):
                            u = unit(sprev, bprev, c)
                            cn = nr * WB
                            ac.wait_ge(sP, u + 1)
                            nc.scalar.activation(
                                kt_ps[:pb, :cn],
                                ps_s[u % NPSUM][:pb, :cn],
                                Act.Exp, scale=EXP_SCALE,
                                accum_out=acc_s[:pb, u:u + 1]).then_inc(sA, 1)
            # trailing batch
            pb = G * len(cfg.batches[NB - 1])
            for c, (ro, nr) in enumerate(cfg.rchunks):
                u = unit(NSLAB - 1, NB - 1, c)
                cn = nr * WB
                ac.wait_ge(sP, u + 1)
                nc.scalar.activation(
                    kt_ps[:pb, :cn], ps_s[u % NPSUM][:pb, :cn],
                    Act.Exp, scale=EXP_SCALE,
                    accum_out=acc_s[:pb, u:u + 1]).then_inc(sA, 1)
    return nc


# ---------------- host side ----------------

def _block_q(plane, cfg, fill=0.0):
    """[H, W] -> flat blocked+haloed [G*Hp*WBH], borders filled with `fill`."""
    p = np.full((cfg.Hp, cfg.W + 2 * R), fill, dtype=np.float32)
    p[R:R + cfg.H, R:R + cfg.W] = plane
    out = np.empty((G, cfg.Hp, cfg.WBH), dtype=np.float32)
    for g in range(G):
        out[g] = p[:, g * cfg.WB:g * cfg.WB + cfg.WBH]
    return np.ascontiguousarray(out).reshape(-1)


def host_precompute(rgb, depth, depth_gt, depth_mask, depth_gt_mask,
                    xy1_grid, Ts, cfg, b):
    tb = b ^ 1
    xy1 = np.asarray(xy1_grid[b], np.float32)
    dep = np.asarray(depth[b, 0], np.float32)
    dgt_b = np.asarray(depth_gt[b, 0], np.float32)
    dgt_t = np.asarray(depth_gt[tb, 0], np.float32)
    mp = np.asarray(depth_mask[b, 0], np.float32)
    mg_b = np.asarray(depth_gt_mask[b, 0], np.float32)
    mg_t = np.asarray(depth_gt_mask[tb, 0], np.float32)

    xyz_p = xy1 * dep
    T21 = (np.linalg.inv(np.asarray(Ts[tb], np.float64)) @
           np.asarray(Ts[b], np.float64)).astype(np.float32)
    Rm, tv = T21[:3, :3], T21[:3, 3]
    txyz = np.einsum('ij,jhw->ihw', Rm, xyz_p).astype(np.float32) \
        + tv[:, None, None].astype(np.float32)
    pos = (txyz[2] > 0).astype(np.float32) * mp

    rgb_b = np.asarray(rgb[b], np.float32)
    rgb_t = np.asarray(rgb[tb], np.float32)
    # quantize rgb first so the norm channels match the fp16 device products
    q16 = lambda x: x.astype(np.float16).astype(np.float32)
    rgb_bq = q16(rgb_b)
    rgb_tq = q16(rgb_t)
    Q2c = (rgb_bq ** 2).sum(0)                    # query rgb norm (frame b)
    R2c_b = (rgb_bq ** 2).sum(0)
    R2c_t = (rgb_tq ** 2).sum(0)

    qa = np.empty((2, CA, cfg.NQ), np.float32)
    ra = np.empty((2, CA, cfg.NQ), np.float32)
    qb = np.empty((2, CB, cfg.NQ), np.float32)
    rb = np.empty((2, CB, cfg.NQ), np.float32)
    for c in range(3):
        qa[0, c] = _block_q(xyz_p[c], cfg)
        qa[1, c] = _block_q(txyz[c], cfg)
        ra[0, c] = _block_q(xy1[c] * dgt_b, cfg)
        ra[1, c] = _block_q(xy1[c] * dgt_t, cfg)
        qb[0, c] = _block_q(rgb_b[c], cfg)
        qb[1, c] = qb[0, c]
        rb[0, c] = _block_q(rgb_b[c], cfg)
        rb[1, c] = _block_q(rgb_t[c], cfg)
    qa[:, 3] = 0.0
    ra[0, 3] = _block_q(np.sqrt(R2c_b + MSK_C * (1.0 - mg_b)), cfg)
    ra[1, 3] = _block_q(np.sqrt(R2c_t + MSK_C * (1.0 - mg_t)), cfg)
    qb[0, 3] = _block_q(Q2c + MSK_C * (1.0 - mp), cfg, fill=MSK_C)
    qb[1, 3] = _block_q(Q2c + MSK_C * (1.0 - pos), cfg, fill=MSK_C)
    rb[:, 3] = _block_q(np.ones_like(mp), cfg, fill=1.0)
    return {"qa_d": qa.astype(np.float16), "ra_d": ra.astype(np.float16),
            "qb_d": qb.astype(np.float16), "rb_d": rb.astype(np.float16),
            "selA_d": make_selA(), "selB_d": make_selB()}


def make_in_maps(rgb, depth, depth_gt, depth_mask, depth_gt_mask, xy1_grid, Ts,
                 cfg, n_cores=8):
    return [host_precompute(rgb, depth, depth_gt, depth_mask, depth_gt_mask,
                            xy1_grid, Ts, cfg, b) for b in range(n_cores)]


_CACHED = {}


def _get_nc(cfg_key=(352, 1216, 32)):
    if cfg_key not in _CACHED:
        cfg = Cfg(*cfg_key)
        nc = bass.Bass()
        emit(nc, cfg)
        _CACHED[cfg_key] = (nc, cfg)
    return _CACHED[cfg_key]


def kernel(rgb, depth, depth_gt, depth_mask, depth_gt_mask, xy1_grid, Ts,
           **run_kwargs):
    from concourse.bass_utils import run_bass_kernel_spmd
    nc, cfg = _get_nc()
    maps = make_in_maps(rgb, depth, depth_gt, depth_mask, depth_gt_mask,
                        xy1_grid, Ts, cfg)
    res = run_bass_kernel_spmd(nc, maps, list(range(8)), **run_kwargs)
    total = np.float64(0.0)
    for r in res.results:
        total += np.float64(r["out_d"][:, 0].sum())
    n_gt = max(np.asarray(depth_gt_mask, np.float64).sum(), 1.0)
    loss = -total / n_gt
    kernel.last_results = res
    return np.float32(loss)


# revision 26
# speedup vs baseline: 1.0057x; 1.0057x over previous
"""C3DLoss kernel for Trainium2 — 8-core batch-parallel, raw-Bass implementation.

Per core = one batch frame b (tgt pairing partner tb = b^1):
    partial = sum over both terms (same-frame, cross-frame), all 25 shifts
              delta in [-2,2]^2, all pixels p of
        mref(p) * mq(p+delta) * exp(-50*(|xyz_r(p)-xyz_q(p+d)|^2
                                         + |rgb_r(p)-rgb_q(p+d)|^2))
    loss = -(sum of partials) / max(sum(depth_gt_mask), 1)

Device mapping (fp16 hybrid: squared-diff xyz + dot-expansion rgb):
  - |rgb_r - rgb_q|^2 is expanded as R2 + Q2 - 2*R.Q so the rgb side needs
    only a PRODUCT per shift (no subtract+square): cancellation is safe
    because rgb magnitudes are <=1 (fp16 products quantize to ~2e-4).
    xyz (magnitudes up to ~10) stays on the subtract-then-square path.
  - Host pre-blocks planes into G=32 W-blocks with +-2 halo ([G, Hp, WBH],
    fp16).  Partitions = (channel, block); shifts are flat free-dim offsets.
  - Channels (all fp16; per term t):
      A tile (sq-diff, sel weight +1):
        ref   [xg, yg, zg, sqrt(R2c + 400*(1-mg))]
        query [xq, yq, zq, 0]
      B tile (dot, sel weights -2,-2,-2,+1):
        ref   [r, g, b, 1]
        query [r, g, b, Q2c + 400*(1-mq)]   (halo fill 400 kills oob pairs)
    PSUM accumulates  |dxyz|^2 + R2c+400(1-mg) - 2*R.Q + Q2c+400(1-mq)
      = d2 + masks;  ACT applies exp(-50 * psum) with accum_out.
  - Engine split per (term,shift) x 32-row slab [flat free = 1344]:
      DVE:    all subs dA = rA - qA[off] and products prodB = rB * qB[off],
              as paired 2-row APs (two same-term shifts per instruction,
              stride-0 broadcast on the ref operand; fp16 dual-pump
              ~0.58ns/elem), plus the first square-pair on every 5th batch
      ACT:    remaining squares sqA = Square(dA) in pairs, exp chunks
              (junk output to the 8th PSUM bank) with accum_out partials
      PE:     per chunk: selA@sqA (start) + selB@prodB (stop) into
              32-partition PSUM slots, halo cols skipped by strided rhs
      GPSIMD: memsets only -- its slow software elementwise streaming
              degrades every other engine's SBUF access ~2x
  - Batch-granular semaphores (4 shift-slots per sync group) with updates
    embedded via then_inc keep each engine's ops back-to-back
    (per-instruction updates serialize the DVE pipeline, +60%/op).
    12-deep wrap-guarded rotation buffers decouple the DVE<->ACT
    ping-pong; input slabs are triple-buffered fp16 (~27KB moved per
    (slot,slab) per partition, half the fp32 equivalent).
"""

import sys

for _p in ("/opt/trn_rl_repo", "/opt/pypackages"):
    if _p not in sys.path:
        sys.path.insert(0, _p)

from contextlib import ExitStack

import numpy as np

import concourse.bass as bass
import concourse.mybir as mybir
from concourse.ap import AP
from concourse.alu_op_type import AluOpType

F32 = mybir.dt.float32
F16 = mybir.dt.float16

R = 2
G = 32           # W-blocks; one shift-slot = 32 partitions (PE quadrant)
CA = 4           # tile A channels: x, y, z, sqrt(LF)
CB = 4           # tile B channels: r, g, b, LG
SBATCH = 4       # shift slots per 128-partition PSUM bank
NPSUM = 7        # rotating PSUM banks (8th bank = exp junk output)
NSQ = 12         # rotating sq/prod buffers (per side)
NDA = 12         # rotating diff buffers
MSK_C = 400.0
EXP_SCALE = -50.0


class Cfg:
    def __init__(self, H=352, W=1216, HS=32):
        assert W % G == 0 and H % HS == 0
        self.H, self.W, self.HS = H, W, HS
        self.WB = W // G
        self.WBH = self.WB + 2 * R
        self.Hp = H + 2 * R
        self.NSLAB = H // HS
        self.NQ = G * self.Hp * self.WBH     # haloed plane elems
        self.QF = (HS + 2 * R) * self.WBH    # query tile free size (1512)
        self.SF = HS * self.WBH              # slab tile free size (1344)
        cr = max(1, 512 // self.WB)
        self.rchunks = []
        o = 0
        while o < HS:
            self.rchunks.append((o, min(cr, HS - o)))
            o += cr
        self.NC = len(self.rchunks)          # 3 (13,13,6 rows)
        self.slots = [(t, dy, dx) for t in (0, 1)
                      for dy in range(-R, R + 1) for dx in range(-R, R + 1)]
        self.NSLOT = len(self.slots)         # 50
        self.batches = [self.slots[i:i + SBATCH]
                        for i in range(0, len(self.slots), SBATCH)]
        self.NB = len(self.batches)          # 13
        self.UPS = self.NB * self.NC         # units per slab (39)
        self.n_acc = self.NSLAB * self.UPS   # 429

    def batch_of_slot(self, Jg):
        return (Jg // self.NSLOT) * self.NB + (Jg % self.NSLOT) // SBATCH

    def slot_end(self, s, b):
        return s * self.NSLOT + min(SBATCH * (b + 1), self.NSLOT)


def _apv(t_ap, p0, pcnt, free_dims, free_off=0):
    pstride = t_ap.ap[0][0]
    base = t_ap.offset + p0 * pstride + free_off
    return AP(t_ap.tensor, base, [[pstride, pcnt]] + [list(d) for d in free_dims])


def _dram_ap(handle, offset, dims):
    a = handle[:]
    return AP(a.tensor, a.offset + offset, [list(d) for d in dims])


def make_selA():
    s = np.zeros((CA * G, G), dtype=np.float16)
    for c in range(CA):
        for g in range(G):
            s[c * G + g, g] = 1.0
    return s


def make_selB():
    s = np.zeros((CB * G, G), dtype=np.float16)
    for c in range(CB):
        w = -2.0 if c < 3 else 1.0
        for g in range(G):
            s[c * G + g, g] = w
    return s


def emit(nc: bass.Bass, cfg: Cfg):
    HS, WB, WBH = cfg.HS, cfg.WB, cfg.WBH
    NQ, QF, SF = cfg.NQ, cfg.QF, cfg.SF
    NSLAB, NB, NC = cfg.NSLAB, cfg.NB, cfg.NC
    NSLOT = cfg.NSLOT
    Act = mybir.ActivationFunctionType

    dp = nc.declare_dram_parameter
    qa_d = dp("qa_d", [2, CA, NQ], F16, isOutput=False)   # query A (haloed)
    ra_d = dp("ra_d", [2, CA, NQ], F16, isOutput=False)   # ref A (haloed)
    qb_d = dp("qb_d", [2, CB, NQ], F16, isOutput=False)   # query B (haloed)
    rb_d = dp("rb_d", [2, CB, NQ], F16, isOutput=False)   # ref B (haloed)
    selA_d = dp("selA_d", [CA * G, G], F16, isOutput=False)
    selB_d = dp("selB_d", [CB * G, G], F16, isOutput=False)
    out_d = dp("out_d", [128, 1], F32, isOutput=True)
    dbg_d = dp("dbg_d", [128, cfg.n_acc], F32, isOutput=True)

    LD = 8            # load DMAs per slab

    def unit(s, b, c):
        return s * cfg.UPS + b * NC + c

    with ExitStack() as ex:
        E = ex.enter_context
        qa_s = [[E(nc.sbuf_tensor(f"qa{t}{p}", [CA * G, QF + 4], F16))
                 for p in range(3)] for t in range(2)]
        ra_s = [[E(nc.sbuf_tensor(f"ra{t}{p}", [CA * G, SF], F16))
                 for p in range(3)] for t in range(2)]
        qb_s = [[E(nc.sbuf_tensor(f"qb{t}{p}", [CB * G, QF + 4], F16))
                 for p in range(3)] for t in range(2)]
        rb_s = [[E(nc.sbuf_tensor(f"rb{t}{p}", [CB * G, SF], F16))
                 for p in range(3)] for t in range(2)]
        da_t = E(nc.sbuf_tensor("dat", [CA * G, NDA * SF], F16))
        sq_t = E(nc.sbuf_tensor("sqt", [CA * G, NSQ * SF], F16))
        pr_t = E(nc.sbuf_tensor("prt", [CB * G, NSQ * SF], F16))
        acc_s = E(nc.sbuf_tensor("acc", [128, cfg.n_acc], F32))
        res_s = E(nc.sbuf_tensor("res", [128, 1], F32))
        selA_s = E(nc.sbuf_tensor("selA", [CA * G, G], F16))
        selB_s = E(nc.sbuf_tensor("selB", [CB * G, G], F16))
        ps_s = [E(nc.psum_tensor(f"ps{i}", [128, 512], F32))
                for i in range(NPSUM)]
        kt_ps = E(nc.psum_tensor("ktps", [128, 512], F32))

        sLC = E(nc.semaphore("sLC"))  # constant loads
        sL0 = E(nc.semaphore("sL0"))  # even-slab loads
        sL1 = E(nc.semaphore("sL1"))  # odd-slab loads
        sL2 = E(nc.semaphore("sL2"))  # third-phase loads
        sG = E(nc.semaphore("sG"))   # gpsimd memset done
        sV = E(nc.semaphore("sV"))    # DVE batch group done (subs+sq+mults)
        sVs = E(nc.semaphore("sVs"))  # DVE subs done per batch
        sF = E(nc.semaphore("sF"))    # DVE final reduce
        sA1 = E(nc.semaphore("sA1"))  # ACT sq group per batch
        sP = E(nc.semaphore("sP"))   # PE per unit
        sA = E(nc.semaphore("sA"))   # ACT exp per unit
        sL = E(nc.semaphore("sL"))   # final output DMAs
        blk = E(nc.Block())

        @blk.gpsimd
        def _(gp):
            gp.memset(acc_s.ap(), 0.0)
            gp.memset(res_s.ap(), 0.0)
            gp.drain()
            gp.sem_inc(sG, 8)

        @blk.sync
        def _(sp):
            sp.dma_start(selA_s[:], selA_d[:]).then_inc(sLC, 16)
            sp.dma_start(selB_s[:], selB_d[:]).then_inc(sLC, 16)
            for s in range(NSLAB):
                ph = s % 3
                if s >= 3:
                    sp.wait_ge(sV, NB * (s - 2))
                r0 = s * HS
                sLs = (sL0, sL1, sL2)[s % 3]
                for t in range(2):
                    sp.dma_start(
                        _apv(qa_s[t][ph].ap(), 0, CA * G, [[1, QF]], 2),
                        _dram_ap(qa_d, t * CA * NQ + r0 * WBH,
                                 [[NQ, CA], [cfg.Hp * WBH, G], [1, QF]])
                    ).then_inc(sLs, 16)
                    sp.dma_start(
                        ra_s[t][ph].ap(),
                        _dram_ap(ra_d, t * CA * NQ + (r0 + R) * WBH,
                                 [[NQ, CA], [cfg.Hp * WBH, G], [1, SF]])
                    ).then_inc(sLs, 16)
                    sp.dma_start(
                        _apv(qb_s[t][ph].ap(), 0, CB * G, [[1, QF]], 2),
                        _dram_ap(qb_d, t * CB * NQ + r0 * WBH,
                                 [[NQ, CB], [cfg.Hp * WBH, G], [1, QF]])
                    ).then_inc(sLs, 16)
                    sp.dma_start(
                        rb_s[t][ph].ap(),
                        _dram_ap(rb_d, t * CB * NQ + (r0 + R) * WBH,
                                 [[NQ, CB], [cfg.Hp * WBH, G], [1, SF]])
                    ).then_inc(sLs, 16)
            # final output
            sp.wait_ge(sF, 1)
            sp.dma_start(out_d[:], res_s.ap()).then_inc(sL, 16)
            sp.dma_start(dbg_d[:], acc_s.ap()).then_inc(sL, 16)

        @blk.vector
        def _(ve):
            for s in range(NSLAB):
                ph = s % 3
                sLs = (sL0, sL1, sL2)[s % 3]
                ve.wait_ge(sLs, 16 * LD * (s // 3 + 1))
                for b, bslots in enumerate(cfg.batches):
                    nb = len(bslots)
                    lastJ = cfg.slot_end(s, b) - 1
                    if lastJ >= NDA:
                        bold = cfg.batch_of_slot(lastJ - NDA)
                        ve.wait_ge(sA1, bold + 1)
                    if lastJ >= NSQ:
                        bold = cfg.batch_of_slot(lastJ - NSQ)
                        ve.wait_ge(sP, NC * (bold + 1))
                    J0 = s * NSLOT + b * SBATCH
                    Bg2 = s * NB + b
                    # greedy same-term pairs of consecutive slots
                    prs = []
                    i = 0
                    while i < nb:
                        if (i + 1 < nb and bslots[i][0] == bslots[i + 1][0]
                                and (J0 + i) % NDA != NDA - 1
                                and (J0 + i) % NSQ != NSQ - 1):
                            prs.append((i, 2))
                            i += 2
                        else:
                            prs.append((i, 1))
                            i += 1
                    offs = [2 + (R + dy) * WBH + dx for (_, dy, dx) in bslots]
                    last = None
                    for (i, cnt) in prs:
                        t = bslots[i][0]
                        Jg = J0 + i
                        if cnt == 2:
                            dlt = offs[i + 1] - offs[i]
                            last = nc.vector.tensor_tensor(
                                _apv(da_t.ap(), 0, CA * G,
                                     [[SF, 2], [1, SF]], (Jg % NDA) * SF),
                                _apv(ra_s[t][ph].ap(), 0, CA * G,
                                     [[0, 2], [1, SF]]),
                                _apv(qa_s[t][ph].ap(), 0, CA * G,
                                     [[dlt, 2], [1, SF]], offs[i]),
                                AluOpType.subtract)
                        else:
                            last = nc.vector.tensor_tensor(
                                _apv(da_t.ap(), 0, CA * G,
                                     [[1, SF]], (Jg % NDA) * SF),
                                ra_s[t][ph].ap(),
                                _apv(qa_s[t][ph].ap(), 0, CA * G,
                                     [[1, SF]], offs[i]),
                                AluOpType.subtract)
                    last.then_inc(sVs, 1)
                    if Bg2 % 6 == 0 and nb == SBATCH:
                        nc.vector.tensor_mul(
                            _apv(sq_t.ap(), 0, CA * G,
                                 [[SF, 2], [1, SF]], (J0 % NSQ) * SF),
                            _apv(da_t.ap(), 0, CA * G,
                                 [[SF, 2], [1, SF]], (J0 % NDA) * SF),
                            _apv(da_t.ap(), 0, CA * G,
                                 [[SF, 2], [1, SF]], (J0 % NDA) * SF))
                    for (i, cnt) in prs:
                        t = bslots[i][0]
                        Jg = J0 + i
                        if cnt == 2:
                            dlt = offs[i + 1] - offs[i]
                            last = nc.vector.tensor_mul(
                                _apv(pr_t.ap(), 0, CB * G,
                                     [[SF, 2], [1, SF]], (Jg % NSQ) * SF),
                                _apv(rb_s[t][ph].ap(), 0, CB * G,
                                     [[0, 2], [1, SF]]),
                                _apv(qb_s[t][ph].ap(), 0, CB * G,
                                     [[dlt, 2], [1, SF]], offs[i]))
                        else:
                            last = nc.vector.tensor_mul(
                                _apv(pr_t.ap(), 0, CB * G,
                                     [[1, SF]], (Jg % NSQ) * SF),
                                rb_s[t][ph].ap(),
                                _apv(qb_s[t][ph].ap(), 0, CB * G,
                                     [[1, SF]], offs[i]))
                    last.then_inc(sV, 1)
            # final reduction of acc columns
            ve.wait_ge(sA, cfg.n_acc)
            nc.vector.tensor_reduce(
                res_s.ap(), acc_s.ap(), axis=mybir.AxisListType.X,
                op=AluOpType.add).then_inc(sF, 1)

        @blk.tensor
        def _(pe):
            pe.wait_ge(sLC, 32)
            for s in range(NSLAB):
                for b, bslots in enumerate(cfg.batches):
                    Bg = s * NB + b
                    pe.wait_ge(sV, Bg + 1)
                    pe.wait_ge(sA1, Bg + 1)
                    for c, (ro, nr) in enumerate(cfg.rchunks):
                        u = unit(s, b, c)
                        if u >= NPSUM:
                            pe.wait_ge(sA, u - NPSUM + 1)
                        pt = ps_s[u % NPSUM]
                        cn = nr * WB
                        for j in range(len(bslots)):
                            Jg = s * NSLOT + b * SBATCH + j
                            last = (j == len(bslots) - 1)
                            nc.tensor.matmul(
                                pt[G * j:G * (j + 1), :cn], selA_s[:],
                                _apv(sq_t.ap(), 0, CA * G,
                                     [[WBH, nr], [1, WB]],
                                     (Jg % NSQ) * SF + ro * WBH + R),
                                start=True, stop=False, skip_group_check=True,
                                tile_position=(0, G * j))
                            mm = nc.tensor.matmul(
                                pt[G * j:G * (j + 1), :cn], selB_s[:],
                                _apv(pr_t.ap(), 0, CB * G,
                                     [[WBH, nr], [1, WB]],
                                     (Jg % NSQ) * SF + ro * WBH + R),
                                start=False, stop=True, skip_group_check=True,
                                tile_position=(0, G * j))
                            if last:
                                mm.then_inc(sP, 1)

        @blk.scalar
        def _(ac):
            ac.wait_ge(sG, 1)
            for s in range(NSLAB):
                for b in range(NB):
                    Bg = s * NB + b
                    lastJ = cfg.slot_end(s, b) - 1
                    if lastJ >= NSQ:
                        bold = cfg.batch_of_slot(lastJ - NSQ)
                        ac.wait_ge(sP, NC * (bold + 1))
                    ac.wait_ge(sVs, Bg + 1)
                    J0 = s * NSLOT + b * SBATCH
                    nbt = len(cfg.batches[b])
                    sq_pairs = []
                    i = 2 if (Bg % 6 == 0 and nbt == SBATCH) else 0
                    while i < nbt:
                        rem = nbt - i
                        if rem >= 4 and (J0 + i) % NSQ <= NSQ - 4 \
                                and (J0 + i) % NDA <= NDA - 4:
                            sq_pairs.append((J0 + i, 4))
                            i += 4
                        elif rem >= 2 and (J0 + i) % NSQ != NSQ - 1 \
                                and (J0 + i) % NDA != NDA - 1:
                            sq_pairs.append((J0 + i, 2))
                            i += 2
                        else:
                            sq_pairs.append((J0 + i, 1))
                            i += 1
                    for pi, (Jg, pc) in enumerate(sq_pairs):
                        mm = nc.scalar.square(
                            _apv(sq_t.ap(), 0, CA * G,
                                 [[SF, pc], [1, SF]], (Jg % NSQ) * SF),
                            _apv(da_t.ap(), 0, CA * G,
                                 [[SF, pc], [1, SF]], (Jg % NDA) * SF))
                        if pi == len(sq_pairs) - 1:
                            mm.then_inc(sA1, 1)
                    # exps for the previous batch
                    bprev = b - 1
                    sprev = s
                    if b == 0:
                        sprev, bprev = s - 1, NB - 1
                    if sprev >= 0:
                        pb = G * len(cfg.batches[bprev])
                        for c, (ro, nr) in enumerate(cfg.rchunks):
                            u = unit(sprev, bprev, c)
                            cn = nr * WB
                            ac.wait_ge(sP, u + 1)
                            nc.scalar.activation(
                                kt_ps[:pb, :cn],
                                ps_s[u % NPSUM][:pb, :cn],
                                Act.Exp, scale=EXP_SCALE,
                                accum_out=acc_s[:pb, u:u + 1]).then_inc(sA, 1)
            # trailing batch
            pb = G * len(cfg.batches[NB - 1])
            for c, (ro, nr) in enumerate(cfg.rchunks):
                u = unit(NSLAB - 1, NB - 1, c)
                cn = nr * WB
                ac.wait_ge(sP, u + 1)
                nc.scalar.activation(
                    kt_ps[:pb, :cn], ps_s[u % NPSUM][:pb, :cn],
                    Act.Exp, scale=EXP_SCALE,
                    accum_out=acc_s[:pb, u:u + 1]).then_inc(sA, 1)
    return nc


# ---------------- host side ----------------

def _block_q(plane, cfg, fill=0.0):
    """[H, W] -> flat blocked+haloed [G*Hp*WBH], borders filled with `fill`."""
    p = np.full((cfg.Hp, cfg.W + 2 * R), fill, dtype=np.float32)
    p[R:R + cfg.H, R:R + cfg.W] = plane
    out = np.empty((G, cfg.Hp, cfg.WBH), dtype=np.float32)
    for g in range(G):
        out[g] = p[:, g * cfg.WB:g * cfg.WB + cfg.WBH]
    return np.ascontiguousarray(out).reshape(-1)


def host_precompute(rgb, depth, depth_gt, depth_mask, depth_gt_mask,
                    xy1_grid, Ts, cfg, b):
    tb = b ^ 1
    xy1 = np.asarray(xy1_grid[b], np.float32)
    dep = np.asarray(depth[b, 0], np.float32)
    dgt_b = np.asarray(depth_gt[b, 0], np.float32)
    dgt_t = np.asarray(depth_gt[tb, 0], np.float32)
    mp = np.asarray(depth_mask[b, 0], np.float32)
    mg_b = np.asarray(depth_gt_mask[b, 0], np.float32)
    mg_t = np.asarray(depth_gt_mask[tb, 0], np.float32)

    xyz_p = xy1 * dep
    T21 = (np.linalg.inv(np.asarray(Ts[tb], np.float64)) @
           np.asarray(Ts[b], np.float64)).astype(np.float32)
    Rm, tv = T21[:3, :3], T21[:3, 3]
    txyz = np.einsum('ij,jhw->ihw', Rm, xyz_p).astype(np.float32) \
        + tv[:, None, None].astype(np.float32)
    pos = (txyz[2] > 0).astype(np.float32) * mp

    rgb_b = np.asarray(rgb[b], np.float32)
    rgb_t = np.asarray(rgb[tb], np.float32)
    # quantize rgb first so the norm channels match the fp16 device products
    q16 = lambda x: x.astype(np.float16).astype(np.float32)
    rgb_bq = q16(rgb_b)
    rgb_tq = q16(rgb_t)
    Q2c = (rgb_bq ** 2).sum(0)                    # query rgb norm (frame b)
    R2c_b = (rgb_bq ** 2).sum(0)
    R2c_t = (rgb_tq ** 2).sum(0)

    qa = np.empty((2, CA, cfg.NQ), np.float32)
    ra = np.empty((2, CA, cfg.NQ), np.float32)
    qb = np.empty((2, CB, cfg.NQ), np.float32)
    rb = np.empty((2, CB, cfg.NQ), np.float32)
    for c in range(3):
        qa[0, c] = _block_q(xyz_p[c], cfg)
        qa[1, c] = _block_q(txyz[c], cfg)
        ra[0, c] = _block_q(xy1[c] * dgt_b, cfg)
        ra[1, c] = _block_q(xy1[c] * dgt_t, cfg)
        qb[0, c] = _block_q(rgb_b[c], cfg)
        qb[1, c] = qb[0, c]
        rb[0, c] = _block_q(rgb_b[c], cfg)
        rb[1, c] = _block_q(rgb_t[c], cfg)
    qa[:, 3] = 0.0
    ra[0, 3] = _block_q(np.sqrt(R2c_b + MSK_C * (1.0 - mg_b)), cfg)
    ra[1, 3] = _block_q(np.sqrt(R2c_t + MSK_C * (1.0 - mg_t)), cfg)
    qb[0, 3] = _block_q(Q2c + MSK_C * (1.0 - mp), cfg, fill=MSK_C)
    qb[1, 3] = _block_q(Q2c + MSK_C * (1.0 - pos), cfg, fill=MSK_C)
    rb[:, 3] = _block_q(np.ones_like(mp), cfg, fill=1.0)
    return {"qa_d": qa.astype(np.float16), "ra_d": ra.astype(np.float16),
            "qb_d": qb.astype(np.float16), "rb_d": rb.astype(np.float16),
            "selA_d": make_selA(), "selB_d": make_selB()}


def make_in_maps(rgb, depth, depth_gt, depth_mask, depth_gt_mask, xy1_grid, Ts,
                 cfg, n_cores=8):
    return [host_precompute(rgb, depth, depth_gt, depth_mask, depth_gt_mask,
                            xy1_grid, Ts, cfg, b) for b in range(n_cores)]


_CACHED = {}


def _get_nc(cfg_key=(352, 1216, 32)):
    if cfg_key not in _CACHED:
        cfg = Cfg(*cfg_key)
        nc = bass.Bass()
        emit(nc, cfg)
        _CACHED[cfg_key] = (nc, cfg)
    return _CACHED[cfg_key]


def kernel(rgb, depth, depth_gt, depth_mask, depth_gt_mask, xy1_grid, Ts,
           **run_kwargs):
    from concourse.bass_utils import run_bass_kernel_spmd
    nc, cfg = _get_nc()
    maps = make_in_maps(rgb, depth, depth_gt, depth_mask, depth_gt_mask,
                        xy1_grid, Ts, cfg)
    res = run_bass_kernel_spmd(nc, maps, list(range(8)), **run_kwargs)
    total = np.float64(0.0)
    for r in res.results:
        total += np.float64(r["out_d"][:, 0].sum())
    n_gt = max(np.asarray(depth_gt_mask, np.float64).sum(), 1.0)
    loss = -total / n_gt
    kernel.last_results = res
    return np.float32(loss)


# revision 27
# speedup vs baseline: 1.0348x; 1.0290x over previous
"""C3DLoss kernel for Trainium2 — 8-core batch-parallel, raw-Bass implementation.

Per core = one batch frame b (tgt pairing partner tb = b^1):
    partial = sum over both terms (same-frame, cross-frame), all 25 shifts
              delta in [-2,2]^2, all pixels p of
        mref(p) * mq(p+delta) * exp(-50*(|xyz_r(p)-xyz_q(p+d)|^2
                                         + |rgb_r(p)-rgb_q(p+d)|^2))
    loss = -(sum of partials) / max(sum(depth_gt_mask), 1)

Device mapping (fp16 hybrid: squared-diff xyz + dot-expansion rgb):
  - |rgb_r - rgb_q|^2 is expanded as R2 + Q2 - 2*R.Q so the rgb side needs
    only a PRODUCT per shift (no subtract+square): cancellation is safe
    because rgb magnitudes are <=1 (fp16 products quantize to ~2e-4).
    xyz (magnitudes up to ~10) stays on the subtract-then-square path.
  - Host pre-blocks planes into G=32 W-blocks with +-2 halo ([G, Hp, WBH],
    fp16).  Partitions = (channel, block); shifts are flat free-dim offsets.
  - Channels (all fp16; per term t):
      A tile (sq-diff, sel weight +1):
        ref   [xg, yg, zg, sqrt(R2c + 400*(1-mg))]
        query [xq, yq, zq, 0]
      B tile (dot, sel weights -2,-2,-2,+1):
        ref   [r, g, b, 1]
        query [r, g, b, Q2c + 400*(1-mq)]   (halo fill 400 kills oob pairs)
    PSUM accumulates  |dxyz|^2 + R2c+400(1-mg) - 2*R.Q + Q2c+400(1-mq)
      = d2 + masks;  ACT applies exp(-50 * psum) with accum_out.
  - Engine split per (term,shift) x 32-row slab [flat free = 1344]:
      DVE:    all subs dA = rA - qA[off] and products prodB = rB * qB[off],
              as paired 2-row APs (two same-term shifts per instruction,
              stride-0 broadcast on the ref operand; fp16 dual-pump
              ~0.58ns/elem), plus the first square-pair on every 5th batch
      ACT:    remaining squares sqA = Square(dA) in pairs, exp chunks
              (junk output to the 8th PSUM bank) with accum_out partials
      PE:     per chunk: selA@sqA (start) + selB@prodB (stop) into
              32-partition PSUM slots, halo cols skipped by strided rhs
      GPSIMD: memsets only -- its slow software elementwise streaming
              degrades every other engine's SBUF access ~2x
  - Batch-granular semaphores (4 shift-slots per sync group) with updates
    embedded via then_inc keep each engine's ops back-to-back
    (per-instruction updates serialize the DVE pipeline, +60%/op).
    12-deep wrap-guarded rotation buffers decouple the DVE<->ACT
    ping-pong; input slabs are triple-buffered fp16 (~27KB moved per
    (slot,slab) per partition, half the fp32 equivalent).
"""

import sys

for _p in ("/opt/trn_rl_repo", "/opt/pypackages"):
    if _p not in sys.path:
        sys.path.insert(0, _p)

from contextlib import ExitStack

import numpy as np

import concourse.bass as bass
import concourse.mybir as mybir
from concourse.ap import AP
from concourse.alu_op_type import AluOpType

F32 = mybir.dt.float32
F16 = mybir.dt.float16

R = 2
G = 32           # W-blocks; one shift-slot = 32 partitions (PE quadrant)
CA = 4           # tile A channels: x, y, z, sqrt(LF)
CB = 4           # tile B channels: r, g, b, LG
SBATCH = 4       # shift slots per 128-partition PSUM bank
NPSUM = 7        # rotating PSUM banks (8th bank = exp junk output)
NSQ = 12         # rotating sq/prod buffers (per side)
NDA = 12         # rotating diff buffers
MSK_C = 400.0
EXP_SCALE = -50.0


class Cfg:
    def __init__(self, H=352, W=1216, HS=32):
        assert W % G == 0 and H % HS == 0
        self.H, self.W, self.HS = H, W, HS
        self.WB = W // G
        self.WBH = self.WB + 2 * R
        self.Hp = H + 2 * R
        self.NSLAB = H // HS
        self.NQ = G * self.Hp * self.WBH     # haloed plane elems
        self.QF = (HS + 2 * R) * self.WBH    # query tile free size (1512)
        self.SF = HS * self.WBH              # slab tile free size (1344)
        cr = max(1, 512 // self.WB)
        self.rchunks = []
        o = 0
        while o < HS:
            self.rchunks.append((o, min(cr, HS - o)))
            o += cr
        self.NC = len(self.rchunks)          # 3 (13,13,6 rows)
        self.slots = [(t, dy, dx) for t in (0, 1)
                      for dy in range(-R, R + 1) for dx in range(-R, R + 1)]
        self.NSLOT = len(self.slots)         # 50
        self.batches = [self.slots[i:i + SBATCH]
                        for i in range(0, len(self.slots), SBATCH)]
        self.NB = len(self.batches)          # 13
        self.UPS = self.NB * self.NC         # units per slab (39)
        self.n_acc = self.NSLAB * self.UPS   # 429

    def batch_of_slot(self, Jg):
        return (Jg // self.NSLOT) * self.NB + (Jg % self.NSLOT) // SBATCH

    def slot_end(self, s, b):
        return s * self.NSLOT + min(SBATCH * (b + 1), self.NSLOT)


def _apv(t_ap, p0, pcnt, free_dims, free_off=0):
    pstride = t_ap.ap[0][0]
    base = t_ap.offset + p0 * pstride + free_off
    return AP(t_ap.tensor, base, [[pstride, pcnt]] + [list(d) for d in free_dims])


def _dram_ap(handle, offset, dims):
    a = handle[:]
    return AP(a.tensor, a.offset + offset, [list(d) for d in dims])


def make_selA():
    s = np.zeros((CA * G, G), dtype=np.float16)
    for c in range(CA):
        for g in range(G):
            s[c * G + g, g] = 1.0
    return s


def make_selB():
    s = np.zeros((CB * G, G), dtype=np.float16)
    for c in range(CB):
        w = -2.0 if c < 3 else 1.0
        for g in range(G):
            s[c * G + g, g] = w
    return s


def emit(nc: bass.Bass, cfg: Cfg):
    HS, WB, WBH = cfg.HS, cfg.WB, cfg.WBH
    NQ, QF, SF = cfg.NQ, cfg.QF, cfg.SF
    NSLAB, NB, NC = cfg.NSLAB, cfg.NB, cfg.NC
    NSLOT = cfg.NSLOT
    Act = mybir.ActivationFunctionType

    dp = nc.declare_dram_parameter
    qa_d = dp("qa_d", [2, CA, NQ], F16, isOutput=False)   # query A (haloed)
    ra_d = dp("ra_d", [2, CA, NQ], F16, isOutput=False)   # ref A (haloed)
    qb_d = dp("qb_d", [2, CB, NQ], F16, isOutput=False)   # query B (haloed)
    rb_d = dp("rb_d", [2, CB, NQ], F16, isOutput=False)   # ref B (haloed)
    selA_d = dp("selA_d", [CA * G, G], F16, isOutput=False)
    selB_d = dp("selB_d", [CB * G, G], F16, isOutput=False)
    out_d = dp("out_d", [128, 1], F32, isOutput=True)
    dbg_d = dp("dbg_d", [128, cfg.n_acc], F32, isOutput=True)

    LD = 8            # load DMAs per slab

    def unit(s, b, c):
        return s * cfg.UPS + b * NC + c

    with ExitStack() as ex:
        E = ex.enter_context
        qa_s = [[E(nc.sbuf_tensor(f"qa{t}{p}", [CA * G, QF + 4], F16))
                 for p in range(3)] for t in range(2)]
        ra_s = [[E(nc.sbuf_tensor(f"ra{t}{p}", [CA * G, SF], F16))
                 for p in range(3)] for t in range(2)]
        qb_s = [[E(nc.sbuf_tensor(f"qb{t}{p}", [CB * G, QF + 4], F16))
                 for p in range(3)] for t in range(2)]
        rb_s = [[E(nc.sbuf_tensor(f"rb{t}{p}", [CB * G, SF], F16))
                 for p in range(3)] for t in range(2)]
        da_t = E(nc.sbuf_tensor("dat", [CA * G, NDA * SF], F16))
        sq_t = E(nc.sbuf_tensor("sqt", [CA * G, NSQ * SF], F16))
        pr_t = E(nc.sbuf_tensor("prt", [CB * G, NSQ * SF], F16))
        acc_s = E(nc.sbuf_tensor("acc", [128, cfg.n_acc], F32))
        res_s = E(nc.sbuf_tensor("res", [128, 1], F32))
        selA_s = E(nc.sbuf_tensor("selA", [CA * G, G], F16))
        selB_s = E(nc.sbuf_tensor("selB", [CB * G, G], F16))
        ps_s = [E(nc.psum_tensor(f"ps{i}", [128, 512], F32))
                for i in range(NPSUM)]
        kt_ps = E(nc.psum_tensor("ktps", [128, 512], F32))

        sLC = E(nc.semaphore("sLC"))  # constant loads
        sL0 = E(nc.semaphore("sL0"))  # even-slab loads
        sL1 = E(nc.semaphore("sL1"))  # odd-slab loads
        sL2 = E(nc.semaphore("sL2"))  # third-phase loads
        sG = E(nc.semaphore("sG"))   # gpsimd memset done
        sV = E(nc.semaphore("sV"))    # DVE batch group done (subs+sq+mults)
        sVs = E(nc.semaphore("sVs"))  # DVE subs done per batch
        sF = E(nc.semaphore("sF"))    # DVE final reduce
        sA1 = E(nc.semaphore("sA1"))  # ACT sq group per batch
        sP = E(nc.semaphore("sP"))   # PE per unit
        sA = E(nc.semaphore("sA"))   # ACT exp per unit
        sL = E(nc.semaphore("sL"))   # final output DMAs
        blk = E(nc.Block())

        @blk.gpsimd
        def _(gp):
            gp.memset(acc_s.ap(), 0.0)
            gp.memset(res_s.ap(), 0.0)
            gp.drain()
            gp.sem_inc(sG, 8)

        @blk.sync
        def _(sp):
            sp.dma_start(selA_s[:], selA_d[:]).then_inc(sLC, 16)
            sp.dma_start(selB_s[:], selB_d[:]).then_inc(sLC, 16)
            for s in range(NSLAB):
                ph = s % 3
                if s >= 3:
                    sp.wait_ge(sV, NB * (s - 2))
                r0 = s * HS
                sLs = (sL0, sL1, sL2)[s % 3]
                for t in range(2):
                    sp.dma_start(
                        _apv(qa_s[t][ph].ap(), 0, CA * G, [[1, QF]], 2),
                        _dram_ap(qa_d, t * CA * NQ + r0 * WBH,
                                 [[NQ, CA], [cfg.Hp * WBH, G], [1, QF]])
                    ).then_inc(sLs, 16)
                    sp.dma_start(
                        ra_s[t][ph].ap(),
                        _dram_ap(ra_d, t * CA * NQ + (r0 + R) * WBH,
                                 [[NQ, CA], [cfg.Hp * WBH, G], [1, SF]])
                    ).then_inc(sLs, 16)
                    sp.dma_start(
                        _apv(qb_s[t][ph].ap(), 0, CB * G, [[1, QF]], 2),
                        _dram_ap(qb_d, t * CB * NQ + r0 * WBH,
                                 [[NQ, CB], [cfg.Hp * WBH, G], [1, QF]])
                    ).then_inc(sLs, 16)
                    sp.dma_start(
                        rb_s[t][ph].ap(),
                        _dram_ap(rb_d, t * CB * NQ + (r0 + R) * WBH,
                                 [[NQ, CB], [cfg.Hp * WBH, G], [1, SF]])
                    ).then_inc(sLs, 16)
            # final output
            sp.wait_ge(sF, 1)
            sp.dma_start(out_d[:], res_s.ap()).then_inc(sL, 16)
            sp.dma_start(dbg_d[:], acc_s.ap()).then_inc(sL, 16)

        @blk.vector
        def _(ve):
            for s in range(NSLAB):
                ph = s % 3
                sLs = (sL0, sL1, sL2)[s % 3]
                ve.wait_ge(sLs, 16 * LD * (s // 3 + 1))
                for b, bslots in enumerate(cfg.batches):
                    nb = len(bslots)
                    lastJ = cfg.slot_end(s, b) - 1
                    if lastJ >= NDA:
                        bold = cfg.batch_of_slot(lastJ - NDA)
                        ve.wait_ge(sA1, bold + 1)
                    if lastJ >= NSQ:
                        bold = cfg.batch_of_slot(lastJ - NSQ)
                        ve.wait_ge(sP, NC * (bold + 1))
                    J0 = s * NSLOT + b * SBATCH
                    Bg2 = s * NB + b
                    # greedy same-term pairs of consecutive slots
                    prs = []
                    i = 0
                    while i < nb:
                        if (i + 1 < nb and bslots[i][0] == bslots[i + 1][0]
                                and (J0 + i) % NDA != NDA - 1
                                and (J0 + i) % NSQ != NSQ - 1):
                            prs.append((i, 2))
                            i += 2
                        else:
                            prs.append((i, 1))
                            i += 1
                    offs = [2 + (R + dy) * WBH + dx for (_, dy, dx) in bslots]
                    last = None
                    for (i, cnt) in prs:
                        t = bslots[i][0]
                        Jg = J0 + i
                        if cnt == 2:
                            dlt = offs[i + 1] - offs[i]
                            last = nc.vector.tensor_tensor(
                                _apv(da_t.ap(), 0, CA * G,
                                     [[SF, 2], [1, SF]], (Jg % NDA) * SF),
                                _apv(ra_s[t][ph].ap(), 0, CA * G,
                                     [[0, 2], [1, SF]]),
                                _apv(qa_s[t][ph].ap(), 0, CA * G,
                                     [[dlt, 2], [1, SF]], offs[i]),
                                AluOpType.subtract)
                        else:
                            last = nc.vector.tensor_tensor(
                                _apv(da_t.ap(), 0, CA * G,
                                     [[1, SF]], (Jg % NDA) * SF),
                                ra_s[t][ph].ap(),
                                _apv(qa_s[t][ph].ap(), 0, CA * G,
                                     [[1, SF]], offs[i]),
                                AluOpType.subtract)
                    last.then_inc(sVs, 1)
                    if Bg2 % 5 == 0 and nb == SBATCH:
                        nc.vector.tensor_mul(
                            _apv(sq_t.ap(), 0, CA * G,
                                 [[SF, 2], [1, SF]], (J0 % NSQ) * SF),
                            _apv(da_t.ap(), 0, CA * G,
                                 [[SF, 2], [1, SF]], (J0 % NDA) * SF),
                            _apv(da_t.ap(), 0, CA * G,
                                 [[SF, 2], [1, SF]], (J0 % NDA) * SF))
                    for (i, cnt) in prs:
                        t = bslots[i][0]
                        Jg = J0 + i
                        if cnt == 2:
                            dlt = offs[i + 1] - offs[i]
                            last = nc.vector.tensor_mul(
                                _apv(pr_t.ap(), 0, CB * G,
                                     [[SF, 2], [1, SF]], (Jg % NSQ) * SF),
                                _apv(rb_s[t][ph].ap(), 0, CB * G,
                                     [[0, 2], [1, SF]]),
                                _apv(qb_s[t][ph].ap(), 0, CB * G,
                                     [[dlt, 2], [1, SF]], offs[i]))
                        else:
                            last = nc.vector.tensor_mul(
                                _apv(pr_t.ap(), 0, CB * G,
                                     [[1, SF]], (Jg % NSQ) * SF),
                                rb_s[t][ph].ap(),
                                _apv(qb_s[t][ph].ap(), 0, CB * G,
                                     [[1, SF]], offs[i]))
                    last.then_inc(sV, 1)
            # final reduction of acc columns
            ve.wait_ge(sA, cfg.n_acc)
            nc.vector.tensor_reduce(
                res_s.ap(), acc_s.ap(), axis=mybir.AxisListType.X,
                op=AluOpType.add).then_inc(sF, 1)

        @blk.tensor
        def _(pe):
            pe.wait_ge(sLC, 32)
            for s in range(NSLAB):
                for b, bslots in enumerate(cfg.batches):
                    Bg = s * NB + b
                    pe.wait_ge(sV, Bg + 1)
                    pe.wait_ge(sA1, Bg + 1)
                    for c, (ro, nr) in enumerate(cfg.rchunks):
                        u = unit(s, b, c)
                        if u >= NPSUM:
                            pe.wait_ge(sA, u - NPSUM + 1)
                        pt = ps_s[u % NPSUM]
                        cn = nr * WB
                        for j in range(len(bslots)):
                            Jg = s * NSLOT + b * SBATCH + j
                            last = (j == len(bslots) - 1)
                            nc.tensor.matmul(
                                pt[G * j:G * (j + 1), :cn], selA_s[:],
                                _apv(sq_t.ap(), 0, CA * G,
                                     [[WBH, nr], [1, WB]],
                                     (Jg % NSQ) * SF + ro * WBH + R),
                                start=True, stop=False, skip_group_check=True,
                                tile_position=(0, G * j))
                            mm = nc.tensor.matmul(
                                pt[G * j:G * (j + 1), :cn], selB_s[:],
                                _apv(pr_t.ap(), 0, CB * G,
                                     [[WBH, nr], [1, WB]],
                                     (Jg % NSQ) * SF + ro * WBH + R),
                                start=False, stop=True, skip_group_check=True,
                                tile_position=(0, G * j))
                            if last:
                                mm.then_inc(sP, 1)

        @blk.scalar
        def _(ac):
            ac.wait_ge(sG, 1)
            for s in range(NSLAB):
                for b in range(NB):
                    Bg = s * NB + b
                    lastJ = cfg.slot_end(s, b) - 1
                    if lastJ >= NSQ:
                        bold = cfg.batch_of_slot(lastJ - NSQ)
                        ac.wait_ge(sP, NC * (bold + 1))
                    ac.wait_ge(sVs, Bg + 1)
                    J0 = s * NSLOT + b * SBATCH
                    nbt = len(cfg.batches[b])
                    sq_pairs = []
                    i = 2 if (Bg % 5 == 0 and nbt == SBATCH) else 0
                    while i < nbt:
                        rem = nbt - i
                        if rem >= 4 and (J0 + i) % NSQ <= NSQ - 4 \
                                and (J0 + i) % NDA <= NDA - 4:
                            sq_pairs.append((J0 + i, 4))
                            i += 4
                        elif rem >= 2 and (J0 + i) % NSQ != NSQ - 1 \
                                and (J0 + i) % NDA != NDA - 1:
                            sq_pairs.append((J0 + i, 2))
                            i += 2
                        else:
                            sq_pairs.append((J0 + i, 1))
                            i += 1
                    for pi, (Jg, pc) in enumerate(sq_pairs):
                        mm = nc.scalar.square(
                            _apv(sq_t.ap(), 0, CA * G,
                                 [[SF, pc], [1, SF]], (Jg % NSQ) * SF),
                            _apv(da_t.ap(), 0, CA * G,
                                 [[SF, pc], [1, SF]], (Jg % NDA) * SF))
                        if pi == len(sq_pairs) - 1:
                            mm.then_inc(sA1, 1)
                    # exps for the previous batch
                    bprev = b - 1
                    sprev = s
                    if b == 0:
                        sprev, bprev = s - 1, NB - 1
                    if sprev >= 0:
                        pb = G * len(cfg.batches[bprev])
                        for c, (ro, nr) in enumerate(cfg.rchunks):
                            u = unit(sprev, bprev, c)
                            cn = nr * WB
                            ac.wait_ge(sP, u + 1)
                            nc.scalar.activation(
                                kt_ps[:pb, :cn],
                                ps_s[u % NPSUM][:pb, :cn],
                                Act.Exp, scale=EXP_SCALE,
                                accum_out=acc_s[:pb, u:u + 1]).then_inc(sA, 1)
            # trailing batch
            pb = G * len(cfg.batches[NB - 1])
            for c, (ro, nr) in enumerate(cfg.rchunks):
                u = unit(NSLAB - 1, NB - 1, c)
                cn = nr * WB
                ac.wait_ge(sP, u + 1)
                nc.scalar.activation(
                    kt_ps[:pb, :cn], ps_s[u % NPSUM][:pb, :cn],
                    Act.Exp, scale=EXP_SCALE,
                    accum_out=acc_s[:pb, u:u + 1]).then_inc(sA, 1)
    return nc


# ---------------- host side ----------------

def _block_q(plane, cfg, fill=0.0):
    """[H, W] -> flat blocked+haloed [G*Hp*WBH], borders filled with `fill`."""
    p = np.full((cfg.Hp, cfg.W + 2 * R), fill, dtype=np.float32)
    p[R:R + cfg.H, R:R + cfg.W] = plane
    out = np.empty((G, cfg.Hp, cfg.WBH), dtype=np.float32)
    for g in range(G):
        out[g] = p[:, g * cfg.WB:g * cfg.WB + cfg.WBH]
    return np.ascontiguousarray(out).reshape(-1)


def host_precompute(rgb, depth, depth_gt, depth_mask, depth_gt_mask,
                    xy1_grid, Ts, cfg, b):
    tb = b ^ 1
    xy1 = np.asarray(xy1_grid[b], np.float32)
    dep = np.asarray(depth[b, 0], np.float32)
    dgt_b = np.asarray(depth_gt[b, 0], np.float32)
    dgt_t = np.asarray(depth_gt[tb, 0], np.float32)
    mp = np.asarray(depth_mask[b, 0], np.float32)
    mg_b = np.asarray(depth_gt_mask[b, 0], np.float32)
    mg_t = np.asarray(depth_gt_mask[tb, 0], np.float32)

    xyz_p = xy1 * dep
    T21 = (np.linalg.inv(np.asarray(Ts[tb], np.float64)) @
           np.asarray(Ts[b], np.float64)).astype(np.float32)
    Rm, tv = T21[:3, :3], T21[:3, 3]
    txyz = np.einsum('ij,jhw->ihw', Rm, xyz_p).astype(np.float32) \
        + tv[:, None, None].astype(np.float32)
    pos = (txyz[2] > 0).astype(np.float32) * mp

    rgb_b = np.asarray(rgb[b], np.float32)
    rgb_t = np.asarray(rgb[tb], np.float32)
    # quantize rgb first so the norm channels match the fp16 device products
    q16 = lambda x: x.astype(np.float16).astype(np.float32)
    rgb_bq = q16(rgb_b)
    rgb_tq = q16(rgb_t)
    Q2c = (rgb_bq ** 2).sum(0)                    # query rgb norm (frame b)
    R2c_b = (rgb_bq ** 2).sum(0)
    R2c_t = (rgb_tq ** 2).sum(0)

    qa = np.empty((2, CA, cfg.NQ), np.float32)
    ra = np.empty((2, CA, cfg.NQ), np.float32)
    qb = np.empty((2, CB, cfg.NQ), np.float32)
    rb = np.empty((2, CB, cfg.NQ), np.float32)
    for c in range(3):
        qa[0, c] = _block_q(xyz_p[c], cfg)
        qa[1, c] = _block_q(txyz[c], cfg)
        ra[0, c] = _block_q(xy1[c] * dgt_b, cfg)
        ra[1, c] = _block_q(xy1[c] * dgt_t, cfg)
        qb[0, c] = _block_q(rgb_b[c], cfg)
        qb[1, c] = qb[0, c]
        rb[0, c] = _block_q(rgb_b[c], cfg)
        rb[1, c] = _block_q(rgb_t[c], cfg)
    qa[:, 3] = 0.0
    ra[0, 3] = _block_q(np.sqrt(R2c_b + MSK_C * (1.0 - mg_b)), cfg)
    ra[1, 3] = _block_q(np.sqrt(R2c_t + MSK_C * (1.0 - mg_t)), cfg)
    qb[0, 3] = _block_q(Q2c + MSK_C * (1.0 - mp), cfg, fill=MSK_C)
    qb[1, 3] = _block_q(Q2c + MSK_C * (1.0 - pos), cfg, fill=MSK_C)
    rb[:, 3] = _block_q(np.ones_like(mp), cfg, fill=1.0)
    return {"qa_d": qa.astype(np.float16), "ra_d": ra.astype(np.float16),
            "qb_d": qb.astype(np.float16), "rb_d": rb.astype(np.float16),
            "selA_d": make_selA(), "selB_d": make_selB()}


def make_in_maps(rgb, depth, depth_gt, depth_mask, depth_gt_mask, xy1_grid, Ts,
                 cfg, n_cores=8):
    return [host_precompute(rgb, depth, depth_gt, depth_mask, depth_gt_mask,
                            xy1_grid, Ts, cfg, b) for b in range(n_cores)]


_CACHED = {}


def _get_nc(cfg_key=(352, 1216, 32)):
    if cfg_key not in _CACHED:
        cfg = Cfg(*cfg_key)
        nc = bass.Bass()
        emit(nc, cfg)
        _CACHED[cfg_key] = (nc, cfg)
    return _CACHED[cfg_key]


def kernel(rgb, depth, depth_gt, depth_mask, depth_gt_mask, xy1_grid, Ts,
           **run_kwargs):
    from concourse.bass_utils import run_bass_kernel_spmd
    nc, cfg = _get_nc()
    maps = make_in_maps(rgb, depth, depth_gt, depth_mask, depth_gt_mask,
                        xy1_grid, Ts, cfg)
    res = run_bass_kernel_spmd(nc, maps, list(range(8)), **run_kwargs)
    total = np.float64(0.0)
    for r in res.results:
        total += np.float64(r["out_d"][:, 0].sum())
    n_gt = max(np.asarray(depth_gt_mask, np.float64).sum(), 1.0)
    loss = -total / n_gt
    kernel.last_results = res
    return np.float32(loss)
